# revision 1
# baseline (speedup 1.0000x reference)
"""Expert-parallel MoE routing kernel for 8 TRN2 NeuronCores.

softmax(relu(x @ W1[r] + b1[r]) @ W2[r] + b2[r]) per token, where r is the
token's route id.  Tokens are dispatched host-side (sorting by route is part
of sharding), one route per core; each core runs a padded two-layer MLP +
softmax in bf16 matmul / f32 accumulate.
"""

import math

import numpy as np
import ml_dtypes

import concourse.bass as bass
import concourse.mybir as mybir
import concourse.tile as tile
from concourse import bacc
from concourse.bass_utils import run_bass_kernel_spmd

# Problem shape (nn_CategoryRouter): fixed by the grading harness.
B, S, D, F, V, R = 4, 1024, 768, 3072, 2048, 8
N_CORES = 8
KD = D // 128   # 6  K-tiles for layer 1
KF = F // 128   # 24 K-tiles for layer 2 (= M-tiles of layer-1 output)
NV = V // 512   # 4  512-wide output column tiles

BF16 = mybir.dt.bfloat16
F32 = mybir.dt.float32
np_bf16 = ml_dtypes.bfloat16

_CACHE: dict[tuple, object] = {}


def _build(cap: int, use_b2: bool):
    """One-core SPMD graph: [cap,D] tokens through its route's head."""
    full_mt = cap // 128                  # full 128-token tiles
    pm = cap - full_mt * 128              # tail tokens (0, 32, 64 or 96)
    if cap <= 512:
        l1_slices = [(0, cap)]
    else:
        # Balanced split: a tiny second matmul (e.g. 512+32) pays a ~60ns
        # NX dispatch floor; two equal ones stream at N/2.4 each.  Keep both
        # slices >= 257 so each matmul's ~107ns LDWEIGHTS hides under the
        # other's stream time.
        # Balanced split: a tiny second matmul (e.g. 512+32) pays a ~60ns
        # NX dispatch floor; two near-equal slices stream at N/2.4 each
        # (measured best of the 512+32 / 272+256 / 264+264 / 256+272
        # variants).
        s1 = (cap // 2 + 15) // 16 * 16
        l1_slices = [(0, s1), (s1, cap - s1)]
    AF = mybir.ActivationFunctionType

    nc = bacc.Bacc("TRN2", target_bir_lowering=False, debug=False,
                   num_devices=N_CORES)

    xt_d = nc.declare_dram_parameter("xt", [128, KD, cap], BF16, isOutput=False)
    w1_d = nc.declare_dram_parameter("w1", [KF, 128, KD * 128], BF16, isOutput=False)
    b1_d = nc.declare_dram_parameter("b1", [128, KF], F32, isOutput=False)
    w2_d = nc.declare_dram_parameter("w2", [KF, NV // 2, 128, 2, 512], BF16,
                                     isOutput=False)
    b2_d = nc.declare_dram_parameter("b2", [1, V], BF16, isOutput=False)
    out_d = nc.declare_dram_parameter("out", [cap, V], F32, isOutput=True)

    with tile.TileContext(nc) as tc:
        with (
            tc.tile_pool(name="wpool", bufs=1) as wpool,
            tc.tile_pool(name="work", bufs=2) as work,
            tc.tile_pool(name="psum", bufs=8, space="PSUM") as psum,
        ):
            # Warm-up fodder: zeroed tiles for PE HAM ramp + Exp table load,
            # all runnable during the initial DMA fill.
            # Warm-up fodder; memset pinned to vector — gpsimd doesn't run
            # its first instruction until ~6us after NEFF start and would
            # delay the PE ramp.
            wz = wpool.tile([128, 512], BF16, name="wz")
            nc.gpsimd.memset(wz[:], 0.0)
            dummy = work.tile([1, 2], F32, name="dummy", tag="dummy", bufs=1)
            nc.scalar.activation(dummy[:], wz[:1, :2], AF.Exp)
            ps_w = psum.tile([128, 512], F32, name="ps_w", tag="mm", bufs=8)
            n_warm = 8
            for i in range(n_warm):
                nc.tensor.matmul(ps_w[:], lhsT=wz[:, :128], rhs=wz[:],
                                 start=(i == 0), stop=(i == n_warm - 1))

            # Resident inputs, DMA'd once in consumption order: xt + b1
            # (layer-1 eviction needs it), then W1 f-major, then W2 f-major
            # so layer 2's first pass can stream behind the DMA engines.
            # Transfers are spread over the sync HWDGE ring plus SWDGE
            # (gpsimd, otherwise idle) so the fill runs two rings wide.
            # The scalar ring is left free: scalar runs the psum evictions
            # and queued DMA triggers there stall the whole pipeline.
            xt_s = [wpool.tile([128, cap], BF16, name=f"xt_s{k}", tag=f"xt_{k}")
                    for k in range(KD)]
            b1_s = wpool.tile([128, KF], F32, name="b1_s")
            w1_s = [wpool.tile([128, KD * 128], BF16, name=f"w1_s{f}",
                               tag=f"w1_{f}") for f in range(KF)]
            # W2 tiles pair two adjacent v-slices so each fill DMA moves
            # 256KB (better ring throughput than 128KB).
            w2_pairs = [[wpool.tile([128, 2, 512], BF16, name=f"w2_p{f}_{p}",
                                    tag=f"w2_{f}_{p}") for p in range(NV // 2)]
                        for f in range(KF)]
            w2_s = [[w2_pairs[f][v // 2][:, v % 2, :] for v in range(NV)]
                    for f in range(KF)]
            # Per-ring transfer lists, in consumption order; the first L1
            # matmul needs only xt0+w1_0, so those lead the sync ring.
            sync_q = [(xt_s[0], xt_d[:, 0, :]), (w1_s[0], w1_d[0]),
                      (xt_s[2], xt_d[:, 2, :]), (xt_s[4], xt_d[:, 4, :])] + \
                     [(w1_s[f], w1_d[f]) for f in range(2, KF, 2)]
            gp_q = [(xt_s[1], xt_d[:, 1, :]), (xt_s[3], xt_d[:, 3, :]),
                    (xt_s[5], xt_d[:, 5, :]), (b1_s, b1_d[:])] + \
                   [(w1_s[f], w1_d[f]) for f in range(1, KF, 2)]
            for i, (f, p) in enumerate((f, p) for f in range(KF)
                                       for p in range(NV // 2)):
                (sync_q if i % 2 == 0 else gp_q).append(
                    (w2_pairs[f][p], w2_d[f, p]))
            for eng, q in ((nc.sync, sync_q), (nc.gpsimd, gp_q)):
                for dst, src in q:
                    eng.dma_start(out=dst[:], in_=src)
            b2_s = wpool.tile([1, V], BF16, name="b2_s")
            ones = wpool.tile([1, 128], BF16, name="ones")
            if use_b2:
                nc.sync.dma_start(out=b2_s[:], in_=b2_d[:])
                nc.any.memset(ones[:], 1.0)

            # Layer 1: ht[f] = relu(W1[:, f-block].T @ X.T + b1[f-block]),
            # stored [F-part, token] so it feeds layer 2 as lhsT directly.
            # Both cap-slices share each (f,k) weight load back-to-back.
            ht = [wpool.tile([128, cap], BF16, name=f"ht{f}", tag=f"ht_{f}")
                  for f in range(KF)]
            for f in range(KF):
                pss = [psum.tile([128, 512], F32, name=f"ps1_{f}_{o}", tag="mm",
                                 bufs=8) for o, _ in l1_slices]
                for k in range(KD):
                    for ps, (off, sz) in zip(pss, l1_slices):
                        nc.tensor.matmul(
                            ps[:, :sz],
                            lhsT=w1_s[f][:, k * 128:(k + 1) * 128],
                            rhs=xt_s[k][:, off:off + sz],
                            start=(k == 0), stop=(k == KD - 1),
                        )
                for ps, (off, sz) in zip(pss, l1_slices):
                    # relu(x+b1) on the (otherwise idle) vector engine keeps
                    # psum eviction latency off the scalar engine's queue.
                    nc.vector.tensor_scalar(
                        ht[f][:, off:off + sz], ps[:, :sz],
                        b1_s[:, f:f + 1], 0.0,
                        op0=mybir.AluOpType.add, op1=mybir.AluOpType.max)

            # Layer 2 + softmax.  Logits are O(5), so exp without
            # max-subtraction is exact in f32; the row sum is accumulated by
            # the eviction itself.  The v-loop is INSIDE the f-loop so all NV
            # psum banks accumulate in parallel off one ht weight load.
            def l2_block(m_off, m_sz, tag, v_outer=False):
                # v_outer: per-v accumulate+exp chains (more ldweights, but
                # only one v-slice of work remains after the last matmul) —
                # used for the final tile to shorten the kernel tail.
                pss = [psum.tile([128, 512], F32, name=f"ps2_{tag}_{v}",
                                 tag="mm", bufs=8) for v in range(NV)]
                exps = work.tile([128, V], F32, name=f"exps{tag}", tag="exps",
                                 bufs=3)
                sums = work.tile([128, NV], F32, name=f"sums{tag}", tag="sums",
                                 bufs=3)
                loop = ([(f, v) for v in range(NV) for f in range(KF)]
                        if v_outer else
                        [(f, v) for f in range(KF) for v in range(NV)])
                for f, v in loop:
                    nc.tensor.matmul(
                        pss[v][:m_sz, :], lhsT=ht[f][:, m_off:m_off + m_sz],
                        rhs=w2_s[f][v][:],
                        start=(f == 0), stop=(f == KF - 1 and not use_b2),
                    )
                    if f == KF - 1:
                        if use_b2:
                            nc.tensor.matmul(
                                pss[v][:m_sz, :], lhsT=ones[:, :m_sz],
                                rhs=b2_s[:, v * 512:(v + 1) * 512],
                                start=False, stop=True,
                            )
                        nc.scalar.activation(exps[:m_sz, v * 512:(v + 1) * 512],
                                             pss[v][:m_sz, :], AF.Exp,
                                             accum_out=sums[:m_sz, v:v + 1])
                rsum = work.tile([128, 1], F32, name=f"rsum{tag}", tag="rsum",
                                 bufs=3)
                nc.vector.reduce_sum(rsum[:m_sz], sums[:m_sz, :],
                                     axis=mybir.AxisListType.X)
                nc.vector.reciprocal(rsum[:m_sz], rsum[:m_sz])
                for v in range(NV):
                    sl = slice(v * 512, (v + 1) * 512)
                    nc.vector.tensor_scalar_mul(exps[:m_sz, sl], exps[:m_sz, sl],
                                                rsum[:m_sz])
                    if v_outer and v == NV - 1:
                        # Final output chunk of the kernel: split across both
                        # rings so the last DMA is half as long.
                        for h in range(2):
                            hs = slice(v * 512 + h * 256, v * 512 + (h + 1) * 256)
                            [nc.sync, nc.gpsimd][h].dma_start(
                                out=out_d[m_off:m_off + m_sz, hs],
                                in_=exps[:m_sz, hs])
                    else:
                        [nc.sync, nc.gpsimd][v % 2].dma_start(
                            out=out_d[m_off:m_off + m_sz, sl], in_=exps[:m_sz, sl])

            # Pass order: full tile 0 first (its slow f-consumption tolerates
            # the W2 DMA frontier), then the tail tile (so its long softmax
            # chain of cross-partition fixups hides under the remaining full
            # tiles), then the rest; the final full tile runs v-outer for a
            # short kernel tail.
            if full_mt > 0:
                l2_block(0, 128, "0", v_outer=(full_mt == 1))

            # Tail tile, pm in {32, 64}: pack the NV v-slices into psum
            # column groups of width g so the PE streams them concurrently;
            # partition group j of bank b holds v-slice b*ngrp+j of the tail
            # tokens.
            if pm > 64:
                l2_block(full_mt * 128, pm, "t")
            elif pm > 0:
                g = 32 if pm <= 32 else 64
                ngrp = 128 // g          # v-slices per psum bank
                nbank = math.ceil(NV / ngrp)
                groups = [(v, divmod(v, ngrp)) for v in range(NV)]
                t_off = full_mt * 128
                expt = work.tile([128, 512 * nbank], F32, name="expt",
                                 tag="expt", bufs=1)
                sumt = work.tile([128, nbank], F32, name="sumt", tag="sumt",
                                 bufs=1)
                pst = [psum.tile([128, 512], F32, name=f"ps2t_{b}", tag="mm",
                                 bufs=8) for b in range(nbank)]
                for f in range(KF):
                    for v, (b, j) in groups:
                        nc.tensor.matmul(
                            pst[b][j * g:j * g + pm, :],
                            lhsT=ht[f][:, t_off:t_off + pm],
                            rhs=w2_s[f][v][:],
                            start=(f == 0), stop=(f == KF - 1 and not use_b2),
                            tile_position=(0, j * g), skip_group_check=True,
                        )
                if use_b2:
                    for v, (b, j) in groups:
                        nc.tensor.matmul(
                            pst[b][j * g:j * g + pm, :], lhsT=ones[:, :pm],
                            rhs=b2_s[:, v * 512:(v + 1) * 512],
                            start=False, stop=True, tile_position=(0, j * g),
                            skip_group_check=True,
                        )
                for v, (b, j) in groups:
                    nc.scalar.activation(
                        expt[j * g:j * g + pm, b * 512:(b + 1) * 512],
                        pst[b][j * g:j * g + pm, :],
                        AF.Exp, accum_out=sumt[j * g:j * g + pm, b:b + 1])
                # Cross-partition-group row sum: gather the group sums into
                # one partition block via tiny SBUF->SBUF DMAs.
                sum4 = work.tile([128, NV], F32, name="sum4", tag="sum4", bufs=1)
                for v, (b, j) in groups:
                    nc.sync.dma_start(out=sum4[:pm, v:v + 1],
                                      in_=sumt[j * g:j * g + pm, b:b + 1])
                rsut = work.tile([128, 1], F32, name="rsut", tag="rsut", bufs=1)
                nc.vector.reduce_sum(rsut[:pm], sum4[:pm, :],
                                     axis=mybir.AxisListType.X)
                nc.vector.reciprocal(rsut[:pm], rsut[:pm])
                for j in range(1, ngrp):
                    nc.sync.dma_start(out=rsut[j * g:j * g + pm, 0:1],
                                      in_=rsut[:pm, 0:1])
                for v, (b, j) in groups:
                    row = slice(j * g, j * g + pm)
                    nc.vector.tensor_scalar_mul(
                        expt[row, b * 512:(b + 1) * 512],
                        expt[row, b * 512:(b + 1) * 512], rsut[row, 0:1])
                    nc.sync.dma_start(
                        out=out_d[t_off:t_off + pm, v * 512:(v + 1) * 512],
                        in_=expt[row, b * 512:(b + 1) * 512])

            for m in range(1, full_mt):
                l2_block(m * 128, 128, str(m), v_outer=(m == full_mt - 1))

    nc.compile()
    return nc


def _dispatch(e_two, route_ids, W1, b1, W2, b2):
    """Host-side shard: sort tokens by route, pad, tile weights per core."""
    x = np.ascontiguousarray(e_two, dtype=np.float32).reshape(-1, D)
    rid = np.asarray(route_ids).reshape(-1)
    order = np.argsort(rid, kind="stable")
    counts = np.bincount(rid, minlength=R)
    cap = max(128, int(math.ceil(counts.max() / 16)) * 16)

    in_maps, perms = [], []
    start = 0
    for r in range(R):
        n = int(counts[r])
        toks = order[start:start + n]
        start += n
        perms.append(toks)

        xp = np.zeros((cap, D), np.float32)
        xp[:n] = x[toks]
        # [128, KD, cap]: partition p holds feature k*128+p of every token.
        xt = np.ascontiguousarray(
            xp.T.reshape(KD, 128, cap).transpose(1, 0, 2)).astype(np_bf16)
        # [KF, 128, KD*128]: row p of block f holds W1[k*128+p, f*128+m].
        w1 = np.ascontiguousarray(
            np.asarray(W1[r], np.float32).reshape(KD, 128, KF, 128)
            .transpose(2, 1, 0, 3).reshape(KF, 128, KD * 128)).astype(np_bf16)
        b1t = np.ascontiguousarray(
            np.asarray(b1[r], np.float32).reshape(KF, 128).T)
        # [KF, NV/2, 128, 2, 512]: row p of (f,pair,half) holds
        # W2[f*128+p, (2*pair+half)*512+nn].
        w2 = np.ascontiguousarray(
            np.asarray(W2[r], np.float32).reshape(KF, 128, NV // 2, 2, 512)
            .transpose(0, 2, 1, 3, 4)).astype(np_bf16)
        b2t = np.asarray(b2[r], np.float32).reshape(1, V).astype(np_bf16)
        in_maps.append({"xt": xt, "w1": w1, "b1": b1t, "w2": w2, "b2": b2t})
    return in_maps, perms, counts, cap


def kernel(e_two, route_ids, W1, b1, W2, b2):
    in_maps, perms, counts, cap = _dispatch(e_two, route_ids, W1, b1, W2, b2)
    use_b2 = bool(np.any(np.asarray(b2)))

    key = (cap, use_b2)
    nc = _CACHE.get(key)
    if nc is None:
        nc = _build(cap, use_b2)
        _CACHE[key] = nc

    res = run_bass_kernel_spmd(nc, in_maps, core_ids=list(range(N_CORES)))

    out = np.zeros((B * S, V), np.float32)
    for r in range(R):
        out[perms[r]] = res.results[r]["out"][:counts[r]]
    return out.reshape(B, S, V)

